# revision 28
# baseline (speedup 1.0000x reference)
"""Trainium2 kernel for nn_BinaryDiffRow.

Math: y = x @ base_t + (x * coeff) @ S,  S = unpack_signs(mask) in {-1,+1}
Fold: y = x @ W_eff,  W_eff = base_t + coeff[:,None] * S   (single matmul)
      W_eff = (base_t - coeff) + 2*coeff*bit,  bit in {0,1}
      (base_t - coeff folded on host; bit unpacked on device)

Sharding (tensor parallel over output columns, 8 cores):
  core j owns output columns [512j, 512j+512).
  - Builds its W_eff slab (4096 x 512, bf16) once on-device:
    bit-unpack of mask via DVE shift/AND, kept resident in SBUF.
  - Streams all 8192 tokens of x (host-pretransposed, bf16) through the PE,
    accumulating psum[128tok, 512] over 32 k-chunks.
  - Host concatenates the 8 column slabs into the full output.
"""

import os
import sys

import numpy as np

for _p in ("/opt/trn_rl_repo",):
    if _p not in sys.path and os.path.isdir(_p):
        sys.path.insert(0, _p)

import ml_dtypes  # noqa: E402

# --- problem constants (hardcoded per contract) ---
B, S, IN, OUT = 4, 2048, 4096, 4096
NTOK = B * S  # 8192
NCORES = 8
OUT_SH = OUT // NCORES  # 512
P = 128
NBITS = 32



def build_bass(
    in_dim=IN,
    ntok=NTOK,
    out_sh=OUT_SH,
    x_bufs=6,
    ps_bufs=6,
    repeat_phase2=1,
    variant="xstat",
):
    """Build the single-core Bass program (SPMD: all cores run this)."""
    import concourse.mybir as mybir
    import concourse.tile as tile
    from concourse import bacc
    from contextlib import ExitStack

    kc = in_dim // P  # k-chunks
    tt = ntok // P  # token tiles
    nwords = out_sh // NBITS

    # Bacc (not plain Bass): its finalize() runs generate_event_semaphores,
    # which splits multi-sem waits — walrus only allows 1 wait/instruction.
    nc = bacc.Bacc("TRN2")
    dt = mybir.dt
    Alu = mybir.AluOpType

    xt = nc.dram_tensor("xt", (tt, P, kc, P), dt.bfloat16, kind="ExternalInput")
    # host ships (base_t - coeff) pre-tiled to (P, kc, out_sh) in bf16;
    # DMA'd directly into the resident W slab, then the unpacked +/-2c*bit
    # delta is accumulated in place (no per-k DMAs -> no DMA-wait pileups).
    bmc = nc.dram_tensor("bmc", (P, kc, out_sh), dt.bfloat16, kind="ExternalInput")
    # merged int32 const block: [shift table | mask tiled | 2*coeff bits]
    # one DMA -> one semaphore wait for all phase-1 consumers (the 3D-AP
    # TensorTensor encoding only has room for a single sync wait).
    cw = out_sh + kc * nwords + kc
    consts = nc.dram_tensor("consts", (P, cw), dt.int32, kind="ExternalInput")
    y = nc.dram_tensor("y", (ntok, out_sh), dt.float32, kind="ExternalOutput")

    with ExitStack() as ctx:
        tc = ctx.enter_context(tile.TileContext(nc))
        cpool = ctx.enter_context(tc.tile_pool(name="consts", bufs=1))
        wpool = ctx.enter_context(tc.tile_pool(name="w", bufs=1))
        upool = ctx.enter_context(tc.tile_pool(name="unpack", bufs=2))
        xpool = ctx.enter_context(tc.tile_pool(name="x", bufs=x_bufs))
        opool = ctx.enter_context(tc.tile_pool(name="out", bufs=3))
        pspool = ctx.enter_context(tc.tile_pool(name="ps", bufs=ps_bufs, space="PSUM"))

        consts_sb = cpool.tile([P, cw], dt.int32)
        nc.sync.dma_start(consts_sb[:], consts[:, :])
        shifts_sb = consts_sb[:, :out_sh]
        mask_off = out_sh
        c2_off = out_sh + kc * nwords

        # base-coeff staging (bf16) + resident W_eff slab [128, kc, out_sh]
        bmc_sb = cpool.tile([P, kc, out_sh], dt.bfloat16)
        nc.sync.dma_start(bmc_sb[:], bmc[:, :, :])
        w_sb = wpool.tile([P, kc, out_sh], dt.bfloat16)

        # Sacrificial 2D copies: absorb DMA semaphore waits into the DVE's
        # vector clock, so TensorTensor instructions (1 wait slot only) never
        # need to carry a DMA wait on top of a slot wait.
        warm = cpool.tile([P, 2], dt.int32)
        nc.vector.tensor_copy(warm[:, 0:1], consts_sb[:, :1])
        nc.vector.tensor_copy(warm[:, 1:2], bmc_sb[:, 0, :1].bitcast(dt.int16))

        # ---- phase 1: unpack mask + fold into W_eff ----
        def phase1():
            for k in range(kc):
                # sh = word_{o//32} >> (o%32)
                sh_t = upool.tile([P, out_sh], dt.int32, tag="sh")
                mask_k = consts_sb[
                    :, mask_off + k * nwords : mask_off + (k + 1) * nwords
                ]
                nc.vector.tensor_tensor(
                    sh_t[:],
                    mask_k[:, :, None].to_broadcast((P, nwords, NBITS)),
                    shifts_sb[:],
                    Alu.logical_shift_right,
                )
                c2_col = consts_sb[:, c2_off + k : c2_off + k + 1].bitcast(dt.float32)
                bit_t = upool.tile([P, out_sh], dt.int32, tag="bit")
                nc.vector.tensor_scalar(bit_t[:], sh_t[:], 1, None, Alu.bitwise_and)
                # d = 2c * bit  (ACT engine: scale-multiply with i32->f32 cast,
                # offloads work from the DVE which is the phase-1 bottleneck)
                d_t = upool.tile([P, out_sh], dt.float32, tag="d")
                nc.scalar.activation(
                    d_t[:], bit_t[:], mybir.ActivationFunctionType.Copy, scale=c2_col
                )
                # W[k] = (base - c) + d
                nc.vector.tensor_tensor(w_sb[:, k, :], d_t[:], bmc_sb[:, k, :], Alu.add)

        # ---- phase 2: stream tokens through the resident W_eff ----
        def phase2():
            for t in range(tt):
                x_sb = xpool.tile([P, kc, P], dt.bfloat16, tag="x")
                nc.sync.dma_start(x_sb[:], xt[t])
                ps = pspool.tile([P, out_sh], dt.float32, tag="ps")
                for k in range(kc):
                    nc.tensor.matmul(
                        ps[:],
                        lhsT=x_sb[:, k, :],
                        rhs=w_sb[:, k, :],
                        start=(k == 0),
                        stop=(k == kc - 1),
                    )
                o_sb = opool.tile([P, out_sh], dt.float32, tag="o")
                nc.vector.tensor_copy(o_sb[:], ps[:])
                nc.sync.dma_start(y[t * P : (t + 1) * P, :], o_sb[:])

        if repeat_phase2 == 1:
            phase1()
            phase2()
        else:
            # benchmarking only: repeat the whole (idempotent) kernel body in
            # a HW loop so one NEFF execution amortizes the ~85ms axon
            # dispatch overhead
            with tc.For_i(0, repeat_phase2, 1):
                phase1()
                phase2()

    nc.finalize()  # Bacc: reg alloc + event-sem wait splitting
    return nc


def make_in_maps(x, base_t, coeff, mask, in_dim=IN, ntok=NTOK, out_sh=OUT_SH, ncores=NCORES):
    kc = in_dim // P
    tt = ntok // P
    nwords = out_sh // NBITS

    x2d = np.ascontiguousarray(x.reshape(-1, in_dim))
    xT = np.ascontiguousarray(x2d.T).astype(ml_dtypes.bfloat16)  # (in, ntok)
    # (k,p,t,c) -> (t,p,k,c): per token tile, per partition, k-chunks contiguous
    xt_tiled = np.ascontiguousarray(xT.reshape(kc, P, tt, P).transpose(2, 1, 0, 3))

    coeff = coeff.astype(np.float32)
    c2 = np.ascontiguousarray((2.0 * coeff).reshape(kc, P).T)  # (P, kc) f32
    shifts = np.broadcast_to(
        np.tile(np.arange(NBITS, dtype=np.int32), nwords), (P, out_sh)
    )

    bmc_full = base_t.astype(np.float32) - coeff[:, None]  # (in, out)

    in_maps = []
    for j in range(ncores):
        # (kc, P, out_sh) -> (P, kc, out_sh), bf16
        bmc_j = np.ascontiguousarray(
            bmc_full[:, j * out_sh : (j + 1) * out_sh]
            .reshape(kc, P, out_sh)
            .transpose(1, 0, 2)
            .astype(ml_dtypes.bfloat16)
        )
        # mask slab tiled to [p, k*nwords+w]
        m_j = (
            mask[:, j * nwords : (j + 1) * nwords]
            .reshape(kc, P, nwords)
            .transpose(1, 0, 2)
            .reshape(P, kc * nwords)
            .astype(np.int32)
        )
        consts = np.concatenate(
            [shifts, m_j, c2.view(np.int32)], axis=1
        ).astype(np.int32)
        in_maps.append(
            {
                "xt": xt_tiled,
                "bmc": bmc_j,
                "consts": np.ascontiguousarray(consts),
            }
        )
    return in_maps


_CACHED = {}


def kernel(x, base_t, coeff, mask):
    from concourse.bass_utils import run_bass_kernel_spmd

    if "nc" not in _CACHED:
        _CACHED["nc"] = build_bass()
    nc = _CACHED["nc"]

    x = np.asarray(x, dtype=np.float32)
    base_t = np.asarray(base_t, dtype=np.float32)
    coeff = np.asarray(coeff, dtype=np.float32)
    mask = np.asarray(mask, dtype=np.int32)
    in_maps = make_in_maps(x, base_t, coeff, mask)
    res = run_bass_kernel_spmd(nc, in_maps, core_ids=list(range(NCORES)))
    outs = res.results
    y = np.concatenate([outs[j]["y"] for j in range(NCORES)], axis=1)
    y = y.reshape(B, S, OUT).astype(np.float32)
    return y


if __name__ == "__main__":
    # smoke test at full size
    rng = np.random.default_rng(0)
    x = rng.standard_normal((B, S, IN), dtype=np.float32)
    base_t = (rng.standard_normal((IN, OUT), dtype=np.float32) * 0.02).astype(np.float32)
    coeff = (rng.random(IN, dtype=np.float32) * 0.01).astype(np.float32)
    mask = rng.integers(0, 2**31 - 1, size=(IN, OUT // NBITS), dtype=np.int32)
    y = kernel(x=x, base_t=base_t, coeff=coeff, mask=mask)
    print("y", y.shape, y.dtype)



# revision 32
# speedup vs baseline: 1.2516x; 1.2516x over previous
"""Trainium2 kernel for nn_BinaryDiffRow.

Math: y = x @ base_t + (x * coeff) @ S,  S = unpack_signs(mask) in {-1,+1}
Fold: y = x @ W_eff,  W_eff = base_t + coeff[:,None] * S   (single matmul)
      W_eff = (base_t - coeff) + 2*coeff*bit,  bit in {0,1}
      (base_t - coeff folded on host; bit unpacked on device)

Sharding (tensor parallel over output columns, 8 cores):
  core j owns output columns [512j, 512j+512).
  - Builds its W_eff slab (4096 x 512, bf16) once on-device:
    bit-unpack of mask via DVE shift/AND, kept resident in SBUF.
  - Streams all 8192 tokens of x (host-pretransposed, bf16) through the PE,
    accumulating psum[128tok, 512] over 32 k-chunks.
  - Host concatenates the 8 column slabs into the full output.
"""

import os
import sys

import numpy as np

for _p in ("/opt/trn_rl_repo",):
    if _p not in sys.path and os.path.isdir(_p):
        sys.path.insert(0, _p)

import ml_dtypes  # noqa: E402

# --- problem constants (hardcoded per contract) ---
B, S, IN, OUT = 4, 2048, 4096, 4096
NTOK = B * S  # 8192
NCORES = 8
OUT_SH = OUT // NCORES  # 512
P = 128
NBITS = 32



def build_bass(
    in_dim=IN,
    ntok=NTOK,
    out_sh=OUT_SH,
    x_bufs=6,
    ps_bufs=6,
    repeat_phase2=1,
    loop_phases="both",  # "both" | "p2" — what the benchmark For_i wraps
    p1_act=True,  # offload the scale-cast to ACT (False: all-DVE phase 1)
):
    """Build the single-core Bass program (SPMD: all cores run this)."""
    import concourse.mybir as mybir
    import concourse.tile as tile
    from concourse import bacc
    from contextlib import ExitStack

    kc = in_dim // P  # k-chunks
    tt = ntok // P  # token tiles
    nwords = out_sh // NBITS

    # Bacc (not plain Bass): its finalize() runs generate_event_semaphores,
    # which splits multi-sem waits — walrus only allows 1 wait/instruction.
    nc = bacc.Bacc("TRN2")
    dt = mybir.dt
    Alu = mybir.AluOpType

    xt = nc.dram_tensor("xt", (tt, P, kc, P), dt.bfloat16, kind="ExternalInput")
    # host ships (base_t - coeff) pre-tiled to (P, kc, out_sh) in bf16;
    # DMA'd directly into the resident W slab, then the unpacked +/-2c*bit
    # delta is accumulated in place (no per-k DMAs -> no DMA-wait pileups).
    bmc = nc.dram_tensor("bmc", (P, kc, out_sh), dt.bfloat16, kind="ExternalInput")
    # merged int32 const block: [shift table | mask tiled | 2*coeff bits]
    # one DMA -> one semaphore wait for all phase-1 consumers (the 3D-AP
    # TensorTensor encoding only has room for a single sync wait).
    cw = out_sh + kc * nwords + kc
    consts = nc.dram_tensor("consts", (P, cw), dt.int32, kind="ExternalInput")
    y = nc.dram_tensor("y", (ntok, out_sh), dt.float32, kind="ExternalOutput")

    with ExitStack() as ctx:
        tc = ctx.enter_context(tile.TileContext(nc))
        cpool = ctx.enter_context(tc.tile_pool(name="consts", bufs=1))
        wpool = ctx.enter_context(tc.tile_pool(name="w", bufs=1))
        upool = ctx.enter_context(tc.tile_pool(name="unpack", bufs=2))
        xpool = ctx.enter_context(tc.tile_pool(name="x", bufs=x_bufs))
        opool = ctx.enter_context(tc.tile_pool(name="out", bufs=3))
        pspool = ctx.enter_context(tc.tile_pool(name="ps", bufs=ps_bufs, space="PSUM"))

        consts_sb = cpool.tile([P, cw], dt.int32)
        nc.sync.dma_start(consts_sb[:], consts[:, :])
        shifts_sb = consts_sb[:, :out_sh]
        mask_off = out_sh
        c2_off = out_sh + kc * nwords

        # base-coeff staging (bf16) + resident W_eff slab [128, kc, out_sh]
        bmc_sb = cpool.tile([P, kc, out_sh], dt.bfloat16)
        nc.sync.dma_start(bmc_sb[:], bmc[:, :, :])
        w_sb = wpool.tile([P, kc, out_sh], dt.bfloat16)

        # Sacrificial 2D copies: absorb DMA semaphore waits into the DVE's
        # vector clock, so TensorTensor instructions (1 wait slot only) never
        # need to carry a DMA wait on top of a slot wait.
        warm = cpool.tile([P, 2], dt.int32)
        nc.vector.tensor_copy(warm[:, 0:1], consts_sb[:, :1])
        nc.vector.tensor_copy(warm[:, 1:2], bmc_sb[:, 0, :1].bitcast(dt.int16))

        # ---- phase 1: unpack mask + fold into W_eff ----
        def phase1():
            for k in range(kc):
                # sh = word_{o//32} >> (o%32)
                sh_t = upool.tile([P, out_sh], dt.int32, tag="sh")
                mask_k = consts_sb[
                    :, mask_off + k * nwords : mask_off + (k + 1) * nwords
                ]
                nc.vector.tensor_tensor(
                    sh_t[:],
                    mask_k[:, :, None].to_broadcast((P, nwords, NBITS)),
                    shifts_sb[:],
                    Alu.logical_shift_right,
                )
                c2_col = consts_sb[:, c2_off + k : c2_off + k + 1].bitcast(dt.float32)
                bit_t = upool.tile([P, out_sh], dt.int32, tag="bit")
                nc.vector.tensor_scalar(bit_t[:], sh_t[:], 1, None, Alu.bitwise_and)
                # d = 2c * bit  (scale-multiply with i32->f32 cast)
                d_t = upool.tile([P, out_sh], dt.float32, tag="d")
                if p1_act:
                    # on ACT: offloads work from the DVE (phase-1 bottleneck)
                    nc.scalar.activation(
                        d_t[:], bit_t[:], mybir.ActivationFunctionType.Copy, scale=c2_col
                    )
                else:
                    nc.vector.tensor_scalar(d_t[:], bit_t[:], c2_col, None, Alu.mult)
                # W[k] = (base - c) + d
                nc.vector.tensor_tensor(w_sb[:, k, :], d_t[:], bmc_sb[:, k, :], Alu.add)

        # ---- phase 2: stream tokens through the resident W_eff ----
        def phase2():
            for t in range(tt):
                x_sb = xpool.tile([P, kc, P], dt.bfloat16, tag="x")
                nc.sync.dma_start(x_sb[:], xt[t])
                ps = pspool.tile([P, out_sh], dt.float32, tag="ps")
                for k in range(kc):
                    nc.tensor.matmul(
                        ps[:],
                        lhsT=x_sb[:, k, :],
                        rhs=w_sb[:, k, :],
                        start=(k == 0),
                        stop=(k == kc - 1),
                    )
                o_sb = opool.tile([P, out_sh], dt.float32, tag="o")
                nc.vector.tensor_copy(o_sb[:], ps[:])
                nc.sync.dma_start(y[t * P : (t + 1) * P, :], o_sb[:])

        if repeat_phase2 == 1:
            phase1()
            phase2()
        elif loop_phases == "p2":
            phase1()
            with tc.For_i(0, repeat_phase2, 1):
                phase2()
        else:
            # benchmarking only: repeat the whole (idempotent) kernel body in
            # a HW loop so one NEFF execution amortizes the ~85ms axon
            # dispatch overhead
            with tc.For_i(0, repeat_phase2, 1):
                phase1()
                phase2()

    nc.finalize()  # Bacc: reg alloc + event-sem wait splitting
    return nc


def make_in_maps(x, base_t, coeff, mask, in_dim=IN, ntok=NTOK, out_sh=OUT_SH, ncores=NCORES):
    kc = in_dim // P
    tt = ntok // P
    nwords = out_sh // NBITS

    x2d = np.ascontiguousarray(x.reshape(-1, in_dim))
    xT = np.ascontiguousarray(x2d.T).astype(ml_dtypes.bfloat16)  # (in, ntok)
    # (k,p,t,c) -> (t,p,k,c): per token tile, per partition, k-chunks contiguous
    xt_tiled = np.ascontiguousarray(xT.reshape(kc, P, tt, P).transpose(2, 1, 0, 3))

    coeff = coeff.astype(np.float32)
    c2 = np.ascontiguousarray((2.0 * coeff).reshape(kc, P).T)  # (P, kc) f32
    shifts = np.broadcast_to(
        np.tile(np.arange(NBITS, dtype=np.int32), nwords), (P, out_sh)
    )

    bmc_full = base_t.astype(np.float32) - coeff[:, None]  # (in, out)

    in_maps = []
    for j in range(ncores):
        # (kc, P, out_sh) -> (P, kc, out_sh), bf16
        bmc_j = np.ascontiguousarray(
            bmc_full[:, j * out_sh : (j + 1) * out_sh]
            .reshape(kc, P, out_sh)
            .transpose(1, 0, 2)
            .astype(ml_dtypes.bfloat16)
        )
        # mask slab tiled to [p, k*nwords+w]
        m_j = (
            mask[:, j * nwords : (j + 1) * nwords]
            .reshape(kc, P, nwords)
            .transpose(1, 0, 2)
            .reshape(P, kc * nwords)
            .astype(np.int32)
        )
        consts = np.concatenate(
            [shifts, m_j, c2.view(np.int32)], axis=1
        ).astype(np.int32)
        in_maps.append(
            {
                "xt": xt_tiled,
                "bmc": bmc_j,
                "consts": np.ascontiguousarray(consts),
            }
        )
    return in_maps


# ---------------------------------------------------------------------------
# Variant "wstat": W is the stationary operand (y.T output), each (k, oc)
# weight block shared by two 512-token-group matmuls; a post-finalize surgery
# deletes the redundant duplicate Ldweights (folding their semaphore
# increments into the following matmul), halving weight-load cost.
# ---------------------------------------------------------------------------

TG = 512  # tokens per matmul group (wstat)


def build_bass_wstat(in_dim=IN, ntok=NTOK, out_sh=OUT_SH, x_bufs=6, repeat=1):
    import concourse.mybir as mybir
    import concourse.tile as tile
    from concourse import bacc
    from contextlib import ExitStack

    kc = in_dim // P
    ngrp = ntok // TG
    noc = out_sh // P
    nwords = out_sh // NBITS

    nc = bacc.Bacc("TRN2")
    dt = mybir.dt
    Alu = mybir.AluOpType

    xt = nc.dram_tensor("xt", (ngrp, kc, P, TG), dt.bfloat16, kind="ExternalInput")
    bmc = nc.dram_tensor("bmc", (P, kc, out_sh), dt.bfloat16, kind="ExternalInput")
    cw = out_sh + kc * nwords + kc
    consts = nc.dram_tensor("consts", (P, cw), dt.int32, kind="ExternalInput")
    yT = nc.dram_tensor("y", (out_sh, ntok), dt.float32, kind="ExternalOutput")

    with ExitStack() as ctx:
        tc = ctx.enter_context(tile.TileContext(nc))
        cpool = ctx.enter_context(tc.tile_pool(name="consts", bufs=1))
        wpool = ctx.enter_context(tc.tile_pool(name="w", bufs=1))
        upool = ctx.enter_context(tc.tile_pool(name="unpack", bufs=2))
        xpool = ctx.enter_context(tc.tile_pool(name="x", bufs=x_bufs))
        opool = ctx.enter_context(tc.tile_pool(name="out", bufs=4))
        pspool = ctx.enter_context(tc.tile_pool(name="ps", bufs=1, space="PSUM"))

        consts_sb = cpool.tile([P, cw], dt.int32)
        nc.sync.dma_start(consts_sb[:], consts[:, :])
        shifts_sb = consts_sb[:, :out_sh]
        mask_off = out_sh
        c2_off = out_sh + kc * nwords

        bmc_sb = cpool.tile([P, kc, out_sh], dt.bfloat16)
        nc.sync.dma_start(bmc_sb[:], bmc[:, :, :])
        w_sb = wpool.tile([P, kc, out_sh], dt.bfloat16)

        warm = cpool.tile([P, 2], dt.int32)
        nc.vector.tensor_copy(warm[:, 0:1], consts_sb[:, :1])
        nc.vector.tensor_copy(warm[:, 1:2], bmc_sb[:, 0, :1].bitcast(dt.int16))

        def phase1():
            for k in range(kc):
                sh_t = upool.tile([P, out_sh], dt.int32, tag="sh")
                mask_k = consts_sb[
                    :, mask_off + k * nwords : mask_off + (k + 1) * nwords
                ]
                nc.vector.tensor_tensor(
                    sh_t[:],
                    mask_k[:, :, None].to_broadcast((P, nwords, NBITS)),
                    shifts_sb[:],
                    Alu.logical_shift_right,
                )
                c2_col = consts_sb[:, c2_off + k : c2_off + k + 1].bitcast(dt.float32)
                bit_t = upool.tile([P, out_sh], dt.int32, tag="bit")
                nc.vector.tensor_scalar(bit_t[:], sh_t[:], 1, None, Alu.bitwise_and)
                d_t = upool.tile([P, out_sh], dt.float32, tag="d")
                nc.scalar.activation(
                    d_t[:], bit_t[:], mybir.ActivationFunctionType.Copy, scale=c2_col
                )
                nc.vector.tensor_tensor(w_sb[:, k, :], d_t[:], bmc_sb[:, k, :], Alu.add)

        def phase2():
            for pair in range(ngrp // 2):
                g0, g1 = 2 * pair, 2 * pair + 1
                ps = [
                    [
                        pspool.tile(
                            [P, TG], dt.float32, tag=f"ps{oc}_{gi}",
                            name=f"ps{oc}_{gi}_{pair}",
                        )
                        for gi in range(2)
                    ]
                    for oc in range(noc)
                ]
                for k in range(kc):
                    x0 = xpool.tile([P, TG], dt.bfloat16, tag="x0")
                    nc.sync.dma_start(x0[:], xt[g0, k])
                    x1 = xpool.tile([P, TG], dt.bfloat16, tag="x1")
                    nc.sync.dma_start(x1[:], xt[g1, k])
                    for oc in range(noc):
                        lhsT = w_sb[:, k, oc * P : (oc + 1) * P]
                        nc.tensor.matmul(
                            ps[oc][0][:], lhsT=lhsT, rhs=x0[:],
                            start=(k == 0), stop=(k == kc - 1),
                        )
                        nc.tensor.matmul(
                            ps[oc][1][:], lhsT=lhsT, rhs=x1[:],
                            start=(k == 0), stop=(k == kc - 1),
                        )
                for oc in range(noc):
                    for gi, g in ((0, g0), (1, g1)):
                        o_sb = opool.tile([P, TG], dt.float32, tag="o")
                        nc.vector.tensor_copy(o_sb[:], ps[oc][gi][:])
                        nc.sync.dma_start(
                            yT[oc * P : (oc + 1) * P, g * TG : (g + 1) * TG], o_sb[:]
                        )

        if repeat == 1:
            phase1()
            phase2()
        else:
            with tc.For_i(0, repeat, 1):
                phase1()
                phase2()

    nc.finalize()
    dedupe_ldweights(nc)
    return nc


def dedupe_ldweights(nc):
    """Drop the 2nd of two adjacent identical PE Ldweights. If the redundant
    LDW carries only semaphore updates (no waits), delete it and fold its
    increments into the next PE instruction (cumulative thresholds stay
    correct — waiters observe the tick at the following matmul instead).
    Otherwise replace with a NoOp that keeps the sync_info."""
    import concourse.mybir as mybir

    def wsig(inst):
        return str(inst.ins[0])

    n_del = n_nop = 0
    for fn in nc.m.functions:
        for blk in fn.blocks:
            last_ldw_sig = None
            new_insts = []
            pending_updates = None
            for inst in blk.instructions:
                eng = getattr(inst, "engine", None)
                if eng == mybir.EngineType.PE and pending_updates is not None:
                    si = inst.sync_info
                    if si is None:
                        inst.sync_info = mybir.SyncInfo(
                            on_wait=[], on_update=list(pending_updates)
                        )
                    else:
                        merged = list(si.on_update)
                        for upd in pending_updates:
                            for m in merged:
                                if m.id == upd.id and m.update_mode == upd.update_mode:
                                    m.update_value = m.update_value + upd.update_value
                                    break
                            else:
                                merged.append(upd)
                        si.on_update = merged
                    pending_updates = None
                if eng != mybir.EngineType.PE:
                    new_insts.append(inst)
                    continue
                if isinstance(inst, mybir.InstLdweights):
                    sig = wsig(inst)
                    if sig == last_ldw_sig:
                        si = inst.sync_info
                        waits = list(si.on_wait) if si else []
                        upds = list(si.on_update) if si else []
                        if not waits:
                            if upds:
                                pending_updates = upds
                            n_del += 1
                            continue
                        new_insts.append(
                            mybir.InstNoOp(
                                name=inst.name,
                                engine=mybir.EngineType.PE,
                                ins=[],
                                outs=[],
                                sync_info=inst.sync_info,
                            )
                        )
                        n_nop += 1
                        continue
                    last_ldw_sig = sig
                elif isinstance(inst, mybir.InstMatmult):
                    if getattr(inst, "ldweights", False):
                        last_ldw_sig = None
                new_insts.append(inst)
            assert pending_updates is None, "trailing folded updates lost"
            blk.instructions[:] = new_insts
    return n_del, n_nop


def make_in_maps_wstat(x, base_t, coeff, mask, ncores=NCORES):
    kc = IN // P
    ngrp = NTOK // TG
    nwords = OUT_SH // NBITS

    x2d = np.ascontiguousarray(x.reshape(-1, IN))
    xT = np.ascontiguousarray(x2d.T).astype(ml_dtypes.bfloat16)
    xt_tiled = np.ascontiguousarray(
        xT.reshape(kc, P, ngrp, TG).transpose(2, 0, 1, 3)
    )  # (g, k, p, c)

    coeff = coeff.astype(np.float32)
    c2 = np.ascontiguousarray((2.0 * coeff).reshape(kc, P).T)
    shifts = np.broadcast_to(
        np.tile(np.arange(NBITS, dtype=np.int32), nwords), (P, OUT_SH)
    )
    bmc_full = base_t.astype(np.float32) - coeff[:, None]

    in_maps = []
    for j in range(ncores):
        bmc_j = np.ascontiguousarray(
            bmc_full[:, j * OUT_SH : (j + 1) * OUT_SH]
            .reshape(kc, P, OUT_SH)
            .transpose(1, 0, 2)
            .astype(ml_dtypes.bfloat16)
        )
        m_j = (
            mask[:, j * nwords : (j + 1) * nwords]
            .reshape(kc, P, nwords)
            .transpose(1, 0, 2)
            .reshape(P, kc * nwords)
            .astype(np.int32)
        )
        consts = np.concatenate([shifts, m_j, c2.view(np.int32)], axis=1).astype(
            np.int32
        )
        in_maps.append(
            {"xt": xt_tiled, "bmc": bmc_j, "consts": np.ascontiguousarray(consts)}
        )
    return in_maps


# which implementation kernel() uses: "xstat" or "wstat"
VARIANT = "xstat"

_CACHED = {}


def kernel(x, base_t, coeff, mask):
    from concourse.bass_utils import run_bass_kernel_spmd

    x = np.asarray(x, dtype=np.float32)
    base_t = np.asarray(base_t, dtype=np.float32)
    coeff = np.asarray(coeff, dtype=np.float32)
    mask = np.asarray(mask, dtype=np.int32)

    if VARIANT == "wstat":
        if "nc_w" not in _CACHED:
            _CACHED["nc_w"] = build_bass_wstat()
        nc = _CACHED["nc_w"]
        in_maps = make_in_maps_wstat(x, base_t, coeff, mask)
        res = run_bass_kernel_spmd(nc, in_maps, core_ids=list(range(NCORES)))
        outs = res.results
        yT = np.concatenate([outs[j]["y"] for j in range(NCORES)], axis=0)
        return np.ascontiguousarray(yT.T).reshape(B, S, OUT).astype(np.float32)

    if "nc" not in _CACHED:
        _CACHED["nc"] = build_bass()
    nc = _CACHED["nc"]
    in_maps = make_in_maps(x, base_t, coeff, mask)
    res = run_bass_kernel_spmd(nc, in_maps, core_ids=list(range(NCORES)))
    outs = res.results
    y = np.concatenate([outs[j]["y"] for j in range(NCORES)], axis=1)
    return y.reshape(B, S, OUT).astype(np.float32)


if __name__ == "__main__":
    # smoke test at full size
    rng = np.random.default_rng(0)
    x = rng.standard_normal((B, S, IN), dtype=np.float32)
    base_t = (rng.standard_normal((IN, OUT), dtype=np.float32) * 0.02).astype(np.float32)
    coeff = (rng.random(IN, dtype=np.float32) * 0.01).astype(np.float32)
    mask = rng.integers(0, 2**31 - 1, size=(IN, OUT // NBITS), dtype=np.int32)
    y = kernel(x=x, base_t=base_t, coeff=coeff, mask=mask)
    print("y", y.shape, y.dtype)



# revision 34
# speedup vs baseline: 1.3754x; 1.0989x over previous
"""Trainium2 kernel for nn_BinaryDiffRow.

Math: y = x @ base_t + (x * coeff) @ S,  S = unpack_signs(mask) in {-1,+1}
Fold: y = x @ W_eff,  W_eff = base_t + coeff[:,None] * S   (single matmul)
      W_eff = (base_t - coeff) + 2*coeff*bit,  bit in {0,1}
      (base_t - coeff folded on host; bit unpacked on device)

Sharding (tensor parallel over output columns, 8 cores):
  core j owns output columns [512j, 512j+512).
  - Builds its W_eff slab (4096 x 512, bf16) once on-device:
    bit-unpack of mask via DVE shift/AND, kept resident in SBUF.
  - Streams all 8192 tokens of x (host-pretransposed, bf16) through the PE,
    accumulating psum[128tok, 512] over 32 k-chunks.
  - Host concatenates the 8 column slabs into the full output.
"""

import os
import sys

import numpy as np

for _p in ("/opt/trn_rl_repo",):
    if _p not in sys.path and os.path.isdir(_p):
        sys.path.insert(0, _p)

import ml_dtypes  # noqa: E402

# --- problem constants (hardcoded per contract) ---
B, S, IN, OUT = 4, 2048, 4096, 4096
NTOK = B * S  # 8192
NCORES = 8
OUT_SH = OUT // NCORES  # 512
P = 128
NBITS = 32



def build_bass(
    in_dim=IN,
    ntok=NTOK,
    out_sh=OUT_SH,
    x_bufs=2,  # per token-tile tag (4 tags -> 8 x tiles in flight)
    ps_bufs=2,  # per token-tile tag (4 tags x 2 = all 8 PSUM banks)
    repeat_phase2=1,
    loop_phases="both",  # "both" | "p2" — what the benchmark For_i wraps
    p1_act=True,  # offload the scale-cast to ACT (False: all-DVE phase 1)
):
    """Build the single-core Bass program (SPMD: all cores run this)."""
    import concourse.mybir as mybir
    import concourse.tile as tile
    from concourse import bacc
    from contextlib import ExitStack

    kc = in_dim // P  # k-chunks
    tt = ntok // P  # token tiles
    nwords = out_sh // NBITS

    # Bacc (not plain Bass): its finalize() runs generate_event_semaphores,
    # which splits multi-sem waits — walrus only allows 1 wait/instruction.
    nc = bacc.Bacc("TRN2")
    dt = mybir.dt
    Alu = mybir.AluOpType

    xt = nc.dram_tensor("xt", (tt, P, kc, P), dt.bfloat16, kind="ExternalInput")
    # host ships (base_t - coeff) pre-tiled to (P, kc, out_sh) in bf16;
    # DMA'd directly into the resident W slab, then the unpacked +/-2c*bit
    # delta is accumulated in place (no per-k DMAs -> no DMA-wait pileups).
    bmc = nc.dram_tensor("bmc", (P, kc, out_sh), dt.bfloat16, kind="ExternalInput")
    # merged int32 const block: [shift table | mask tiled | 2*coeff bits]
    # one DMA -> one semaphore wait for all phase-1 consumers (the 3D-AP
    # TensorTensor encoding only has room for a single sync wait).
    cw = out_sh + kc * nwords + kc
    consts = nc.dram_tensor("consts", (P, cw), dt.int32, kind="ExternalInput")
    y = nc.dram_tensor("y", (ntok, out_sh), dt.float32, kind="ExternalOutput")

    with ExitStack() as ctx:
        tc = ctx.enter_context(tile.TileContext(nc))
        cpool = ctx.enter_context(tc.tile_pool(name="consts", bufs=1))
        wpool = ctx.enter_context(tc.tile_pool(name="w", bufs=1))
        upool = ctx.enter_context(tc.tile_pool(name="unpack", bufs=2))
        xpool = ctx.enter_context(tc.tile_pool(name="x", bufs=x_bufs))
        opool = ctx.enter_context(tc.tile_pool(name="out", bufs=3))
        pspool = ctx.enter_context(tc.tile_pool(name="ps", bufs=ps_bufs, space="PSUM"))

        consts_sb = cpool.tile([P, cw], dt.int32)
        nc.sync.dma_start(consts_sb[:], consts[:, :])
        shifts_sb = consts_sb[:, :out_sh]
        mask_off = out_sh
        c2_off = out_sh + kc * nwords

        # base-coeff staging (bf16) + resident W_eff slab [128, kc, out_sh]
        bmc_sb = cpool.tile([P, kc, out_sh], dt.bfloat16)
        nc.sync.dma_start(bmc_sb[:], bmc[:, :, :])
        w_sb = wpool.tile([P, kc, out_sh], dt.bfloat16)

        # Sacrificial 2D copies: absorb DMA semaphore waits into the DVE's
        # vector clock, so TensorTensor instructions (1 wait slot only) never
        # need to carry a DMA wait on top of a slot wait.
        warm = cpool.tile([P, 2], dt.int32)
        nc.vector.tensor_copy(warm[:, 0:1], consts_sb[:, :1])
        nc.vector.tensor_copy(warm[:, 1:2], bmc_sb[:, 0, :1].bitcast(dt.int16))

        # ---- phase 1: unpack mask + fold into W_eff ----
        def phase1():
            for k in range(kc):
                # sh = word_{o//32} >> (o%32)
                sh_t = upool.tile([P, out_sh], dt.int32, tag="sh")
                mask_k = consts_sb[
                    :, mask_off + k * nwords : mask_off + (k + 1) * nwords
                ]
                nc.vector.tensor_tensor(
                    sh_t[:],
                    mask_k[:, :, None].to_broadcast((P, nwords, NBITS)),
                    shifts_sb[:],
                    Alu.logical_shift_right,
                )
                c2_col = consts_sb[:, c2_off + k : c2_off + k + 1].bitcast(dt.float32)
                bit_t = upool.tile([P, out_sh], dt.int32, tag="bit")
                nc.vector.tensor_scalar(bit_t[:], sh_t[:], 1, None, Alu.bitwise_and)
                # d = 2c * bit  (scale-multiply with i32->f32 cast)
                d_t = upool.tile([P, out_sh], dt.float32, tag="d")
                if p1_act:
                    # on ACT: offloads work from the DVE (phase-1 bottleneck)
                    nc.scalar.activation(
                        d_t[:], bit_t[:], mybir.ActivationFunctionType.Copy, scale=c2_col
                    )
                else:
                    nc.vector.tensor_scalar(d_t[:], bit_t[:], c2_col, None, Alu.mult)
                # W[k] = (base - c) + d
                nc.vector.tensor_tensor(w_sb[:, k, :], d_t[:], bmc_sb[:, k, :], Alu.add)

        # ---- phase 2: stream tokens through the resident W_eff ----
        # Token tiles are processed in blocks of BLK with per-tile psum tags
        # (BLK tags x ps_bufs slots = all 8 PSUM banks at BLK=4, ps_bufs=2):
        # inside a block the k-loop is innermost-over-tiles, so several open
        # accumulations consume each w[k] as the DVE produces it — the PE
        # keeps busy during phase 1 instead of stalling behind the unpack.
        BLK = 4

        def phase2():
            for b0 in range(0, tt, BLK):
                blk = list(range(b0, min(b0 + BLK, tt)))
                xs, pss = {}, {}
                for t in blk:
                    xs[t] = xpool.tile(
                        [P, kc, P], dt.bfloat16, tag=f"x{t - b0}", name=f"x_{t}"
                    )
                    nc.sync.dma_start(xs[t][:], xt[t])
                    pss[t] = pspool.tile(
                        [P, out_sh], dt.float32, tag=f"ps{t - b0}", name=f"ps_{t}"
                    )
                for k in range(kc):
                    for t in blk:
                        nc.tensor.matmul(
                            pss[t][:],
                            lhsT=xs[t][:, k, :],
                            rhs=w_sb[:, k, :],
                            start=(k == 0),
                            stop=(k == kc - 1),
                        )
                for t in blk:
                    o_sb = opool.tile([P, out_sh], dt.float32, tag="o", name=f"o_{t}")
                    nc.vector.tensor_copy(o_sb[:], pss[t][:])
                    nc.sync.dma_start(y[t * P : (t + 1) * P, :], o_sb[:])

        if repeat_phase2 == 1:
            phase1()
            phase2()
        elif loop_phases == "p2":
            phase1()
            with tc.For_i(0, repeat_phase2, 1):
                phase2()
        else:
            # benchmarking only: repeat the whole (idempotent) kernel body in
            # a HW loop so one NEFF execution amortizes the ~85ms axon
            # dispatch overhead
            with tc.For_i(0, repeat_phase2, 1):
                phase1()
                phase2()

    nc.finalize()  # Bacc: reg alloc + event-sem wait splitting
    return nc


def make_in_maps(x, base_t, coeff, mask, in_dim=IN, ntok=NTOK, out_sh=OUT_SH, ncores=NCORES):
    kc = in_dim // P
    tt = ntok // P
    nwords = out_sh // NBITS

    x2d = np.ascontiguousarray(x.reshape(-1, in_dim))
    xT = np.ascontiguousarray(x2d.T).astype(ml_dtypes.bfloat16)  # (in, ntok)
    # (k,p,t,c) -> (t,p,k,c): per token tile, per partition, k-chunks contiguous
    xt_tiled = np.ascontiguousarray(xT.reshape(kc, P, tt, P).transpose(2, 1, 0, 3))

    coeff = coeff.astype(np.float32)
    c2 = np.ascontiguousarray((2.0 * coeff).reshape(kc, P).T)  # (P, kc) f32
    shifts = np.broadcast_to(
        np.tile(np.arange(NBITS, dtype=np.int32), nwords), (P, out_sh)
    )

    bmc_full = base_t.astype(np.float32) - coeff[:, None]  # (in, out)

    in_maps = []
    for j in range(ncores):
        # (kc, P, out_sh) -> (P, kc, out_sh), bf16
        bmc_j = np.ascontiguousarray(
            bmc_full[:, j * out_sh : (j + 1) * out_sh]
            .reshape(kc, P, out_sh)
            .transpose(1, 0, 2)
            .astype(ml_dtypes.bfloat16)
        )
        # mask slab tiled to [p, k*nwords+w]
        m_j = (
            mask[:, j * nwords : (j + 1) * nwords]
            .reshape(kc, P, nwords)
            .transpose(1, 0, 2)
            .reshape(P, kc * nwords)
            .astype(np.int32)
        )
        consts = np.concatenate(
            [shifts, m_j, c2.view(np.int32)], axis=1
        ).astype(np.int32)
        in_maps.append(
            {
                "xt": xt_tiled,
                "bmc": bmc_j,
                "consts": np.ascontiguousarray(consts),
            }
        )
    return in_maps


# ---------------------------------------------------------------------------
# Variant "wstat": W is the stationary operand (y.T output), each (k, oc)
# weight block shared by two 512-token-group matmuls; a post-finalize surgery
# deletes the redundant duplicate Ldweights (folding their semaphore
# increments into the following matmul), halving weight-load cost.
# ---------------------------------------------------------------------------

TG = 512  # tokens per matmul group (wstat)


def build_bass_wstat(in_dim=IN, ntok=NTOK, out_sh=OUT_SH, x_bufs=6, repeat=1):
    import concourse.mybir as mybir
    import concourse.tile as tile
    from concourse import bacc
    from contextlib import ExitStack

    kc = in_dim // P
    ngrp = ntok // TG
    noc = out_sh // P
    nwords = out_sh // NBITS

    nc = bacc.Bacc("TRN2")
    dt = mybir.dt
    Alu = mybir.AluOpType

    xt = nc.dram_tensor("xt", (ngrp, kc, P, TG), dt.bfloat16, kind="ExternalInput")
    bmc = nc.dram_tensor("bmc", (P, kc, out_sh), dt.bfloat16, kind="ExternalInput")
    cw = out_sh + kc * nwords + kc
    consts = nc.dram_tensor("consts", (P, cw), dt.int32, kind="ExternalInput")
    yT = nc.dram_tensor("y", (out_sh, ntok), dt.float32, kind="ExternalOutput")

    with ExitStack() as ctx:
        tc = ctx.enter_context(tile.TileContext(nc))
        cpool = ctx.enter_context(tc.tile_pool(name="consts", bufs=1))
        wpool = ctx.enter_context(tc.tile_pool(name="w", bufs=1))
        upool = ctx.enter_context(tc.tile_pool(name="unpack", bufs=2))
        xpool = ctx.enter_context(tc.tile_pool(name="x", bufs=x_bufs))
        opool = ctx.enter_context(tc.tile_pool(name="out", bufs=4))
        pspool = ctx.enter_context(tc.tile_pool(name="ps", bufs=1, space="PSUM"))

        consts_sb = cpool.tile([P, cw], dt.int32)
        nc.sync.dma_start(consts_sb[:], consts[:, :])
        shifts_sb = consts_sb[:, :out_sh]
        mask_off = out_sh
        c2_off = out_sh + kc * nwords

        bmc_sb = cpool.tile([P, kc, out_sh], dt.bfloat16)
        nc.sync.dma_start(bmc_sb[:], bmc[:, :, :])
        w_sb = wpool.tile([P, kc, out_sh], dt.bfloat16)

        warm = cpool.tile([P, 2], dt.int32)
        nc.vector.tensor_copy(warm[:, 0:1], consts_sb[:, :1])
        nc.vector.tensor_copy(warm[:, 1:2], bmc_sb[:, 0, :1].bitcast(dt.int16))

        def phase1():
            for k in range(kc):
                sh_t = upool.tile([P, out_sh], dt.int32, tag="sh")
                mask_k = consts_sb[
                    :, mask_off + k * nwords : mask_off + (k + 1) * nwords
                ]
                nc.vector.tensor_tensor(
                    sh_t[:],
                    mask_k[:, :, None].to_broadcast((P, nwords, NBITS)),
                    shifts_sb[:],
                    Alu.logical_shift_right,
                )
                c2_col = consts_sb[:, c2_off + k : c2_off + k + 1].bitcast(dt.float32)
                bit_t = upool.tile([P, out_sh], dt.int32, tag="bit")
                nc.vector.tensor_scalar(bit_t[:], sh_t[:], 1, None, Alu.bitwise_and)
                d_t = upool.tile([P, out_sh], dt.float32, tag="d")
                nc.scalar.activation(
                    d_t[:], bit_t[:], mybir.ActivationFunctionType.Copy, scale=c2_col
                )
                nc.vector.tensor_tensor(w_sb[:, k, :], d_t[:], bmc_sb[:, k, :], Alu.add)

        def phase2():
            for pair in range(ngrp // 2):
                g0, g1 = 2 * pair, 2 * pair + 1
                ps = [
                    [
                        pspool.tile(
                            [P, TG], dt.float32, tag=f"ps{oc}_{gi}",
                            name=f"ps{oc}_{gi}_{pair}",
                        )
                        for gi in range(2)
                    ]
                    for oc in range(noc)
                ]
                for k in range(kc):
                    x0 = xpool.tile([P, TG], dt.bfloat16, tag="x0")
                    nc.sync.dma_start(x0[:], xt[g0, k])
                    x1 = xpool.tile([P, TG], dt.bfloat16, tag="x1")
                    nc.sync.dma_start(x1[:], xt[g1, k])
                    for oc in range(noc):
                        lhsT = w_sb[:, k, oc * P : (oc + 1) * P]
                        nc.tensor.matmul(
                            ps[oc][0][:], lhsT=lhsT, rhs=x0[:],
                            start=(k == 0), stop=(k == kc - 1),
                        )
                        nc.tensor.matmul(
                            ps[oc][1][:], lhsT=lhsT, rhs=x1[:],
                            start=(k == 0), stop=(k == kc - 1),
                        )
                for oc in range(noc):
                    for gi, g in ((0, g0), (1, g1)):
                        o_sb = opool.tile([P, TG], dt.float32, tag="o")
                        nc.vector.tensor_copy(o_sb[:], ps[oc][gi][:])
                        nc.sync.dma_start(
                            yT[oc * P : (oc + 1) * P, g * TG : (g + 1) * TG], o_sb[:]
                        )

        if repeat == 1:
            phase1()
            phase2()
        else:
            with tc.For_i(0, repeat, 1):
                phase1()
                phase2()

    nc.finalize()
    dedupe_ldweights(nc)
    return nc


def dedupe_ldweights(nc):
    """Drop the 2nd of two adjacent identical PE Ldweights. If the redundant
    LDW carries only semaphore updates (no waits), delete it and fold its
    increments into the next PE instruction (cumulative thresholds stay
    correct — waiters observe the tick at the following matmul instead).
    Otherwise replace with a NoOp that keeps the sync_info."""
    import concourse.mybir as mybir

    def wsig(inst):
        return str(inst.ins[0])

    n_del = n_nop = 0
    for fn in nc.m.functions:
        for blk in fn.blocks:
            last_ldw_sig = None
            new_insts = []
            pending_updates = None
            for inst in blk.instructions:
                eng = getattr(inst, "engine", None)
                if eng == mybir.EngineType.PE and pending_updates is not None:
                    si = inst.sync_info
                    if si is None:
                        inst.sync_info = mybir.SyncInfo(
                            on_wait=[], on_update=list(pending_updates)
                        )
                    else:
                        merged = list(si.on_update)
                        for upd in pending_updates:
                            for m in merged:
                                if m.id == upd.id and m.update_mode == upd.update_mode:
                                    m.update_value = m.update_value + upd.update_value
                                    break
                            else:
                                merged.append(upd)
                        si.on_update = merged
                    pending_updates = None
                if eng != mybir.EngineType.PE:
                    new_insts.append(inst)
                    continue
                if isinstance(inst, mybir.InstLdweights):
                    sig = wsig(inst)
                    if sig == last_ldw_sig:
                        si = inst.sync_info
                        waits = list(si.on_wait) if si else []
                        upds = list(si.on_update) if si else []
                        if not waits:
                            if upds:
                                pending_updates = upds
                            n_del += 1
                            continue
                        new_insts.append(
                            mybir.InstNoOp(
                                name=inst.name,
                                engine=mybir.EngineType.PE,
                                ins=[],
                                outs=[],
                                sync_info=inst.sync_info,
                            )
                        )
                        n_nop += 1
                        continue
                    last_ldw_sig = sig
                elif isinstance(inst, mybir.InstMatmult):
                    if getattr(inst, "ldweights", False):
                        last_ldw_sig = None
                new_insts.append(inst)
            assert pending_updates is None, "trailing folded updates lost"
            blk.instructions[:] = new_insts
    return n_del, n_nop


def make_in_maps_wstat(x, base_t, coeff, mask, ncores=NCORES):
    kc = IN // P
    ngrp = NTOK // TG
    nwords = OUT_SH // NBITS

    x2d = np.ascontiguousarray(x.reshape(-1, IN))
    xT = np.ascontiguousarray(x2d.T).astype(ml_dtypes.bfloat16)
    xt_tiled = np.ascontiguousarray(
        xT.reshape(kc, P, ngrp, TG).transpose(2, 0, 1, 3)
    )  # (g, k, p, c)

    coeff = coeff.astype(np.float32)
    c2 = np.ascontiguousarray((2.0 * coeff).reshape(kc, P).T)
    shifts = np.broadcast_to(
        np.tile(np.arange(NBITS, dtype=np.int32), nwords), (P, OUT_SH)
    )
    bmc_full = base_t.astype(np.float32) - coeff[:, None]

    in_maps = []
    for j in range(ncores):
        bmc_j = np.ascontiguousarray(
            bmc_full[:, j * OUT_SH : (j + 1) * OUT_SH]
            .reshape(kc, P, OUT_SH)
            .transpose(1, 0, 2)
            .astype(ml_dtypes.bfloat16)
        )
        m_j = (
            mask[:, j * nwords : (j + 1) * nwords]
            .reshape(kc, P, nwords)
            .transpose(1, 0, 2)
            .reshape(P, kc * nwords)
            .astype(np.int32)
        )
        consts = np.concatenate([shifts, m_j, c2.view(np.int32)], axis=1).astype(
            np.int32
        )
        in_maps.append(
            {"xt": xt_tiled, "bmc": bmc_j, "consts": np.ascontiguousarray(consts)}
        )
    return in_maps


# which implementation kernel() uses: "xstat" or "wstat"
VARIANT = "xstat"

_CACHED = {}


def kernel(x, base_t, coeff, mask):
    from concourse.bass_utils import run_bass_kernel_spmd

    x = np.asarray(x, dtype=np.float32)
    base_t = np.asarray(base_t, dtype=np.float32)
    coeff = np.asarray(coeff, dtype=np.float32)
    mask = np.asarray(mask, dtype=np.int32)

    if VARIANT == "wstat":
        if "nc_w" not in _CACHED:
            _CACHED["nc_w"] = build_bass_wstat()
        nc = _CACHED["nc_w"]
        in_maps = make_in_maps_wstat(x, base_t, coeff, mask)
        res = run_bass_kernel_spmd(nc, in_maps, core_ids=list(range(NCORES)))
        outs = res.results
        yT = np.concatenate([outs[j]["y"] for j in range(NCORES)], axis=0)
        return np.ascontiguousarray(yT.T).reshape(B, S, OUT).astype(np.float32)

    if "nc" not in _CACHED:
        _CACHED["nc"] = build_bass()
    nc = _CACHED["nc"]
    in_maps = make_in_maps(x, base_t, coeff, mask)
    res = run_bass_kernel_spmd(nc, in_maps, core_ids=list(range(NCORES)))
    outs = res.results
    y = np.concatenate([outs[j]["y"] for j in range(NCORES)], axis=1)
    return y.reshape(B, S, OUT).astype(np.float32)


if __name__ == "__main__":
    # smoke test at full size
    rng = np.random.default_rng(0)
    x = rng.standard_normal((B, S, IN), dtype=np.float32)
    base_t = (rng.standard_normal((IN, OUT), dtype=np.float32) * 0.02).astype(np.float32)
    coeff = (rng.random(IN, dtype=np.float32) * 0.01).astype(np.float32)
    mask = rng.integers(0, 2**31 - 1, size=(IN, OUT // NBITS), dtype=np.int32)
    y = kernel(x=x, base_t=base_t, coeff=coeff, mask=mask)
    print("y", y.shape, y.dtype)

